# revision 21
# baseline (speedup 1.0000x reference)
"""Distributed Bass kernel for a 3-layer GCN (BaseGNN) on 8 TRN2 NeuronCores.

Strategy: nodes block-partitioned across 8 cores (12500 each); edges assigned
to the core owning their destination. GCN symmetric norm factorizes
(norm_e = dinv[src]*dinv[dst]): node features are pre-scaled by dinv before
being shared, aggregation is a plain segment-sum, results post-scaled by dinv.

Per conv, each core gathers pre-scaled source features (fp8 by default) from a
replicated DRAM table via dma_gather, segment-sums them with one-hot selector
matmuls on the TensorEngine (aggregate lands feature-major), then applies
transform + residual + bias + LayerNorm + ReLU in feature-major layout.

V2 schedule: table chunks 0..2 of each conv accumulate first; the final chunk
is processed one dst-quarter at a time, interleaved with that quarter's
transform and staging, so each quarter's AllGather launches as early as
possible and overlaps the next conv's gathers. Tables are fp8: the AllGather
moves a compact [rows,128B] payload (1/4 the bf16-padded bytes) and a local
strided DMA expands it into the 256B-stride gather table.
"""
import sys, os, time

sys.path.insert(0, "/opt/trn_rl_repo")
import numpy as np
import ml_dtypes

BF = ml_dtypes.bfloat16
F8 = ml_dtypes.float8_e4m3

# ---------------- problem constants (hardcoded; kernel.py must be standalone)
N, E, B = 100000, 1600000, 64
IN_D, HID, OUT_D = 20, 128, 256
EPS = 1e-5
NCORE = 8
NLOC = N // NCORE            # 12500 real nodes per core
BLK = 128
NBLK = (NLOC + BLK - 1) // BLK       # 98
NLOCP = NBLK * BLK                   # 12544 padded rows per core in tables
NP = NCORE * NLOCP                   # 100352 padded table rows
GT = 8                               # tiles (of 128 edges) per dma_gather call
KFP8 = bool(int(os.environ.get("KFP8", "1")))

F32 = np.float32


def _quarters():
    """Quarter partition of the per-core blocks."""
    nq = min(4, NBLK)
    qblk = [NBLK // nq + (1 if i < NBLK % nq else 0) for i in range(nq)]
    qb0 = [sum(qblk[:i]) for i in range(nq)]
    qrows = [q * BLK for q in qblk]
    chunkrows = [NCORE * r for r in qrows]
    choff = [sum(chunkrows[:i]) for i in range(nq)]
    return nq, qblk, qb0, qrows, chunkrows, choff


def _table_row(n):
    """real global node id -> row in the quarter-major table layout,
    plus (chunk index, row-within-chunk)."""
    nq, qblk, qb0, qrows, chunkrows, choff = _quarters()
    r = n // NLOC
    l = n % NLOC
    b = l // BLK
    q = np.searchsorted(np.asarray(qb0[1:], np.int64), b, side="right")
    qb0a = np.asarray(qb0, np.int64)[q]
    qra = np.asarray(qrows, np.int64)[q]
    cha = np.asarray(choff, np.int64)[q]
    inchunk = r * qra + (l - qb0a * BLK)
    return cha + inchunk, q, inchunk


def build_plan(edge_index):
    """Per-core edge arrays + common (max-over-cores) tile-count table."""
    NCHUNK = _quarters()[0]
    src = np.concatenate([edge_index[0], np.arange(N, dtype=np.int64)])
    dst = np.concatenate([edge_index[1], np.arange(N, dtype=np.int64)])
    core = dst // NLOC
    per_core = []
    counts = np.zeros((NCORE, NCHUNK, NBLK), np.int64)
    for c in range(NCORE):
        m = core == c
        _, chunk, s = _table_row(src[m])
        d = dst[m] - c * NLOC
        block = d // BLK
        order = np.lexsort((block, chunk))
        s, d, chunk, block = s[order], d[order], chunk[order], block[order]
        key = chunk * NBLK + block
        bounds = np.searchsorted(key, np.arange(NCHUNK * NBLK + 1))
        per_core.append((s, d, bounds))
        counts[c] = (bounds[1:] - bounds[:-1]).reshape(NCHUNK, NBLK)
    ntiles = ((counts.max(0) + 127) // 128).astype(np.int64)      # [NCHUNK, NBLK]
    EP = int(ntiles.sum()) * 128
    plans = []
    for c in range(NCORE):
        s, d, bounds = per_core[c]
        idx = np.zeros(EP, np.int64)
        dl = np.full(EP, 300, np.int64)
        pos = 0
        for ck in range(NCHUNK):
            for b in range(NBLK):
                gi = ck * NBLK + b
                lo, hi = bounds[gi], bounds[gi + 1]
                n = hi - lo
                idx[pos:pos + n] = s[lo:hi]
                dl[pos:pos + n] = d[lo:hi] - b * BLK
                pos += int(ntiles[ck, b]) * 128
        # wrap idx: slot i -> [i%16, i//16], replicated x8 on partitions
        w = idx.reshape(-1, 16).T.astype(np.int16)        # [16, EP/16]
        idxw = np.tile(w, (8, 1))                          # [128, EP/16]
        dlw = dl.reshape(-1, 128).T.astype(BF)             # [128, EP/128]
        plans.append({"idx": np.ascontiguousarray(idxw),
                      "dl": np.ascontiguousarray(dlw)})
    return plans, ntiles, EP


# ---------------- device program ----------------------------------------


def build_nc(ntiles):
    from concourse import bass, bacc, tile
    from concourse.tile_rust import add_dep_helper
    from concourse.bass import mybir
    from concourse.alu_op_type import AluOpType as op
    f32, bf16, i16 = mybir.dt.float32, mybir.dt.bfloat16, mybir.dt.int16
    fp8 = mybir.dt.float8e4
    tdt = fp8 if KFP8 else bf16        # table / message / selector dtype
    TW = 256 if KFP8 else 128          # table row width in elements (256B)
    AF = mybir.ActivationFunctionType

    EP = int(ntiles.sum()) * 128
    nc = bacc.Bacc("TRN2", target_bir_lowering=False, num_swdge_queues=4)

    xs_d = nc.declare_dram_parameter("xs", [NP, TW], tdt, isOutput=False)
    idx_d = nc.declare_dram_parameter("idx", [128, EP // 16], i16, isOutput=False)
    dl_d = nc.declare_dram_parameter("dl", [128, EP // 128], bf16, isOutput=False)
    wts_d = nc.declare_dram_parameter("wts", [128, 640], bf16, isOutput=False)
    wc_d = nc.declare_dram_parameter("wconst", [128, 10], f32, isOutput=False)
    misc_d = nc.declare_dram_parameter("misc", [128, 257], bf16, isOutput=False)
    xloc_d = nc.declare_dram_parameter("xloc", [32, NLOC], bf16, isOutput=False)
    dinvf_d = nc.declare_dram_parameter("dinvf", [1, NLOC], bf16, isOutput=False)
    dinvc_d = nc.declare_dram_parameter("dinvc", [128, NBLK], f32, isOutput=False)
    bone_d = nc.declare_dram_parameter("bone", [NLOCP, 64], bf16, isOutput=False)
    ones_d = nc.declare_dram_parameter("onesrow", [1, 128], bf16, isOutput=False)
    recip_d = nc.declare_dram_parameter("recip", [64, 1], f32, isOutput=False)
    out_d = nc.declare_dram_parameter("out", [64, OUT_D], f32, isOutput=True)

    # wts columns
    W2c, W3ac, W3bc, W1c, RWc = (slice(0, 128), slice(128, 256), slice(256, 384),
                                 slice(384, 512), slice(512, 640))
    # wconst columns
    BIAS1, G1, B1, B2c, G2, B2l, B3A, B3B, ONES, EPSC = range(10)

    NQ, QBLK, QB0, QROWS, CHUNKROWS, CHOFF = _quarters()
    NCHUNK = NQ
    MAXCT = int(ntiles.sum(axis=1).max())
    rg = [list(range(NCORE))]

    with tile.TileContext(nc) as tc:
        with (
            tc.tile_pool(name="const", bufs=1) as cpool,
            tc.tile_pool(name="state", bufs=1) as spool,
            tc.tile_pool(name="io", bufs=2) as iopool,
            tc.tile_pool(name="work", bufs=int(os.environ.get("KBUFS", "4"))) as wpool,
            tc.tile_pool(name="stg", bufs=3) as stgpool,
            tc.tile_pool(name="ln", bufs=4) as lnpool,
            tc.tile_pool(name="lns", bufs=2) as lnspool,
            tc.tile_pool(name="main_ps", bufs=2, space="PSUM") as mainps,
            tc.tile_pool(name="st_ps", bufs=1, space="PSUM") as stps,
            tc.tile_pool(name="aux_ps", bufs=4, space="PSUM") as auxps,
            tc.tile_pool(name="pool_ps", bufs=1, space="PSUM") as poolps,
            tc.tile_pool(name="dram", bufs=1, space="DRAM") as dram,
        ):
            # ---- persistent DRAM scratch
            KREP0 = int(os.environ.get("KREP", "1"))
            ccp_in1 = [dram.tile([QROWS[q], 128], tdt, name=f"cc1_{q}")
                       for q in range(NQ)]
            ccp_in2 = [dram.tile([QROWS[q], 128], tdt, name=f"cc2_{q}")
                       for q in range(NQ)]
            if KFP8:
                # compact AllGather landing buffers + strided expand tables
                ago1 = [[dram.tile([CHUNKROWS[q], 128], tdt, addr_space="Shared",
                                   name=f"ag1_{r}_{q}") for q in range(NQ)]
                        for r in range(KREP0)]
                ago2 = [[dram.tile([CHUNKROWS[q], 128], tdt, addr_space="Shared",
                                   name=f"ag2_{r}_{q}") for q in range(NQ)]
                        for r in range(KREP0)]
                p1_fulls = [[dram.tile([CHUNKROWS[q], TW], tdt,
                                       name=f"p1f{r}_{q}") for q in range(NQ)]
                            for r in range(KREP0)]
                p2_fulls = [[dram.tile([CHUNKROWS[q], TW], tdt,
                                       name=f"p2f{r}_{q}") for q in range(NQ)]
                            for r in range(KREP0)]
            else:
                ago1 = ago2 = None
                p1_fulls = [[dram.tile([CHUNKROWS[q], TW], tdt, addr_space="Shared",
                                       name=f"p1f{r}_{q}") for q in range(NQ)]
                            for r in range(KREP0)]
                p2_fulls = [[dram.tile([CHUNKROWS[q], TW], tdt, addr_space="Shared",
                                       name=f"p2f{r}_{q}") for q in range(NQ)]
                            for r in range(KREP0)]
            ccq_in = dram.tile([64, OUT_D], f32)
            ccq_out = dram.tile([64, OUT_D], f32, addr_space="Shared")

            # ---- constants
            wts = cpool.tile([128, 640], bf16)
            wc = cpool.tile([128, 10], f32)
            misc = cpool.tile([128, 257], bf16)
            dinvc = cpool.tile([128, NBLK], f32)
            recip = cpool.tile([64, 1], f32)
            onesr = cpool.tile([1, 128], bf16)
            dinvr = cpool.tile([1, NLOC], bf16)
            nc.sync.dma_start(wts[:], wts_d[:])
            nc.sync.dma_start(wc[:], wc_d[:])
            nc.sync.dma_start(misc[:], misc_d[:])
            nc.sync.dma_start(dinvc[:], dinvc_d[:])
            nc.sync.dma_start(recip[:], recip_d[:])
            nc.sync.dma_start(onesr[:], ones_d[:])
            nc.sync.dma_start(dinvr[:], dinvf_d[:])
            iota = misc[:, 0:128]
            ident = misc[:, 128:256]
            onesb = misc[:, 256:257]          # bf16 ones column

            # ---- persistent SBUF state
            aggT = spool.tile([128, NLOC], bf16)
            h1T = spool.tile([128, NLOC], bf16)
            hXT = spool.tile([128, NLOC], bf16)

            pool_ps = poolps.tile([64, OUT_D], f32)
            qrr = [0]

            def chunk_tiles(conv, ck, b0, b1, tables, dep, cktile, tmap, Fa):
                """Gathers + selectors + matmuls for blocks [b0,b1) of chunk ck.
                cktile: dict with idxt/dlt tiles + chunk tile_base + consumed
                counter shared across quarter slices of the same chunk."""
                idxt, dlt = cktile["idxt"], cktile["dlt"]
                ck_tiles = cktile["ntiles"]
                # tiles needed through end of block range
                need = cktile["pref"][b1]
                t0 = cktile["gathered"]
                while t0 < need:
                    ntc = min(GT, ck_tiles - t0)
                    msg = wpool.tile([128, GT, TW], tdt, tag="msg")
                    gi = nc.gpsimd.dma_gather(
                        msg[:, :ntc, :],
                        tables[ck],
                        idxt[:, 8 * t0: 8 * (t0 + ntc)],
                        ntc * 128, ntc * 128, TW,
                        queue_num=qrr[0] % 4)
                    qrr[0] += 1
                    if dep is not None:
                        add_dep_helper(gi.ins, dep[ck].ins,
                                       reason="table producer -> gather")
                    sel = wpool.tile([128, GT, 128], tdt, tag="sel")
                    nc.vector.tensor_tensor(
                        sel[:, :ntc, :],
                        dlt[:, t0: t0 + ntc].unsqueeze(2)
                        .broadcast_to([128, ntc, 128]),
                        iota.unsqueeze(1).broadcast_to([128, ntc, 128]),
                        op.is_equal)
                    for j in range(ntc):
                        tmap[t0 + j] = (msg, sel, j)
                    t0 += ntc
                cktile["gathered"] = t0
                # block matmuls
                for b in range(b0, b1):
                    nt = int(ntiles[ck, b])
                    if nt == 0:
                        continue
                    toff = cktile["pref"][b]
                    ps = mainps.tile([128, 128], f32, tag="main")
                    w = min(BLK, NLOC - b * BLK)
                    bs = slice(b * BLK, b * BLK + w)
                    if ck > 0:
                        nc.tensor.matmul(ps[0:Fa, 0:w], ident[0:Fa, 0:Fa],
                                         aggT[0:Fa, bs], start=True, stop=False)
                    for j in range(nt):
                        m, s, off = tmap[toff + j]
                        nc.tensor.matmul(ps[0:Fa, :], m[:, off, 0:Fa],
                                         s[:, off, :],
                                         start=(j == 0 and ck == 0),
                                         stop=(j == nt - 1))
                    nc.scalar.activation(aggT[0:Fa, bs], ps[0:Fa, 0:w],
                                         AF.Identity)

            # idx/dl are graph-constant: load once for all chunks/convs/reps
            TILES_ALL = int(ntiles.sum())
            idx_all = cpool.tile([128, 8 * TILES_ALL], i16)
            nc.sync.dma_start(idx_all[:], idx_d[:])
            dl_all = cpool.tile([128, TILES_ALL], bf16)
            nc.sync.dma_start(dl_all[:], dl_d[:])

            def open_chunk(ck, tile_base):
                ck_tiles = int(ntiles[ck].sum())
                idxt = idx_all[:, 8 * tile_base: 8 * (tile_base + ck_tiles)]
                dlt = dl_all[:, tile_base: tile_base + ck_tiles]
                pref = np.concatenate([[0], np.cumsum(ntiles[ck])]).astype(int)
                return {"idxt": idxt, "dlt": dlt, "ntiles": ck_tiles,
                        "pref": pref, "gathered": 0}

            def transform_q(conv, q):
                """transform + bias + residual + LN + relu for quarter q's
                nodes; aggT -> h1T/hXT."""
                Fa = 32 if conv == 1 else 128
                n0 = QB0[q] * BLK
                nend = min(n0 + QROWS[q], NLOC)
                total = nend - n0
                base = -(-total // 8)
                for i in range(8):
                    lo = n0 + i * base
                    LNT = min(base, nend - lo)
                    if LNT <= 0:
                        break
                    sl = slice(lo, lo + LNT)
                    dbc = auxps.tile([128, 512], f32, tag="aux")
                    nc.tensor.matmul(dbc[0:Fa, 0:LNT], onesr[:, 0:Fa],
                                     dinvr[0:1, sl], start=True, stop=True)
                    z = lnpool.tile([128, 512], bf16, tag="z")
                    nc.vector.tensor_tensor(z[0:Fa, 0:LNT], aggT[0:Fa, sl],
                                            dbc[0:Fa, 0:LNT], op.mult)
                    ps = mainps.tile([128, 512], f32, tag="main")
                    if conv == 1:
                        xsl = lnpool.tile([32, 512], bf16, tag="xsl")
                        nc.sync.dma_start(xsl[:, 0:LNT], xloc_d[:, sl])
                        nc.tensor.matmul(ps[:, 0:LNT], wts[0:32, W1c],
                                         z[0:32, 0:LNT], start=True, stop=False)
                        nc.tensor.matmul(ps[:, 0:LNT], wts[0:32, RWc],
                                         xsl[:, 0:LNT], start=False, stop=True)
                    else:
                        nc.tensor.matmul(ps[:, 0:LNT], wts[:, W2c], z[:, 0:LNT],
                                         start=True, stop=False)
                        nc.tensor.matmul(ps[:, 0:LNT], ident, h1T[:, sl],
                                         start=False, stop=True)
                    y = lnpool.tile([128, 512], bf16, tag="y")
                    bcol = wc[:, BIAS1:BIAS1 + 1] if conv == 1 else wc[:, B2c:B2c + 1]
                    nc.scalar.activation(y[:, 0:LNT], ps[:, 0:LNT], AF.Identity,
                                         bias=bcol)
                    y2 = lnpool.tile([128, 512], bf16, tag="y2")
                    nc.scalar.activation(y2[:, 0:LNT], y[:, 0:LNT], AF.Square)
                    st = stps.tile([64, 512], f32, tag="st")
                    nc.tensor.matmul(st[0:1, 0:LNT], onesb[:, 0:1], y[:, 0:LNT],
                                     start=True, stop=True)
                    nc.tensor.matmul(st[32:33, 0:LNT], onesb[:, 0:1],
                                     y2[:, 0:LNT], start=True, stop=True)
                    mu = lnspool.tile([1, 512], f32, tag="mu")
                    nc.vector.tensor_scalar(mu[0:1, 0:LNT], st[0:1, 0:LNT],
                                            1.0 / 128, None, op.mult)
                    m2 = lnspool.tile([1, 512], f32, tag="m2")
                    nc.vector.tensor_tensor(m2[0:1, 0:LNT], mu[0:1, 0:LNT],
                                            mu[0:1, 0:LNT], op.mult)
                    var = lnspool.tile([1, 512], f32, tag="var")
                    nc.vector.scalar_tensor_tensor(var[0:1, 0:LNT],
                                                   st[32:33, 0:LNT], 1.0 / 128,
                                                   m2[0:1, 0:LNT],
                                                   op.mult, op.subtract)
                    sd = lnspool.tile([1, 512], f32, tag="sd")
                    nc.scalar.activation(sd[0:1, 0:LNT], var[0:1, 0:LNT],
                                         AF.Sqrt, bias=wc[0:1, EPSC:EPSC + 1])
                    rstd = lnspool.tile([1, 512], bf16, tag="rstd")
                    with nc.allow_low_precision(reason="rstd bf16 for bf16 matmul lhs"):
                        nc.vector.reciprocal(rstd[0:1, 0:LNT], sd[0:1, 0:LNT])
                    mr = lnspool.tile([1, 512], bf16, tag="mr")
                    nc.vector.tensor_tensor(mr[0:1, 0:LNT], mu[0:1, 0:LNT],
                                            rstd[0:1, 0:LNT], op.mult)
                    bc1 = auxps.tile([128, 512], f32, tag="aux")
                    nc.tensor.matmul(bc1[:, 0:LNT], onesr[:],
                                     rstd[0:1, 0:LNT], start=True, stop=True)
                    bc2 = auxps.tile([128, 512], f32, tag="aux")
                    nc.tensor.matmul(bc2[:, 0:LNT], onesr[:],
                                     mr[0:1, 0:LNT], start=True, stop=True)
                    xc = lnpool.tile([128, 512], bf16, tag="xc")
                    nc.vector.tensor_tensor(xc[:, 0:LNT], y[:, 0:LNT],
                                            bc1[:, 0:LNT], op.mult)
                    xn = lnpool.tile([128, 512], bf16, tag="xn")
                    nc.vector.tensor_tensor(xn[:, 0:LNT], xc[:, 0:LNT],
                                            bc2[:, 0:LNT], op.subtract)
                    gcol = wc[:, G1:G1 + 1] if conv == 1 else wc[:, G2:G2 + 1]
                    lcol = wc[:, B1:B1 + 1] if conv == 1 else wc[:, B2l:B2l + 1]
                    hdst = h1T if conv == 1 else hXT
                    nc.scalar.activation(hdst[:, sl], xn[:, 0:LNT], AF.Relu,
                                         bias=lcol, scale=gcol)

            def emit_q(hsrc, ccp_in, q):
                """Quarter q: transpose h node-major, scale by dinv, stage,
                DMA to the quarter bounce buffer."""
                nb = QBLK[q]
                g0 = 0
                while g0 < nb:
                    gsz = min(5, nb - g0)
                    stage = stgpool.tile([128, 5, 128], tdt, tag="stage")
                    for k in range(gsz):
                        b = QB0[q] + g0 + k
                        w = min(BLK, NLOC - b * BLK)
                        tp = auxps.tile([128, 128], bf16, tag="aux")
                        nc.tensor.transpose(
                            tp[0:w, :], hsrc[:, b * BLK:b * BLK + w], ident)
                        if w < BLK:
                            nc.vector.memset(stage[:, k, :], 0.0)
                        nc.scalar.activation(
                            stage[0:w, k, :], tp[0:w, :], AF.Identity,
                            scale=dinvc[0:w, b:b + 1])
                    sdma = nc.sync.dma_start(
                        ccp_in[q][g0 * BLK:(g0 + gsz) * BLK, :]
                        .rearrange("(k p) f -> p k f", p=128),
                        stage[:, 0:gsz, :])
                    g0 += gsz
                return sdma

            def launch_ag(ccp_in, agos, p_fulls_q, q):
                """AllGather quarter q (dispatch only)."""
                tgt = agos[q] if KFP8 else p_fulls_q[q]
                return nc.gpsimd.collective_compute(
                    "AllGather", op.bypass, replica_groups=rg,
                    ins=[ccp_in[q].opt()], outs=[tgt.opt()])

            def launch_expand(agos, p_fulls_q, q, ag, after):
                """fp8: expand compact AG output into the 256B-stride table.
                `after` is an order-keeping dep (the conv's last stage DMA) so
                the scheduler cannot hoist the expand ahead of this conv's own
                sync-stream DMAs (it underestimates collective duration)."""
                ex = nc.sync.dma_start(p_fulls_q[q][:, 0:128], agos[q][:])
                add_dep_helper(ex.ins, ag.ins, reason="AG -> expand")
                if after is not None:
                    add_dep_helper(ex.ins, after.ins,
                                   reason="keep expand after conv staging")
                return ex

            def conv_pass(conv, tables, dep, hsrc_for_emit, ccp_in, agos,
                          p_out_q):
                """One conv's aggregation, with the final chunk interleaved
                per-quarter with transform+emit. Returns per-quarter producer
                instructions for the next conv's tables (or None for conv 3)."""
                Fa = 32 if conv == 1 else 128
                last = NCHUNK - 1
                tile_base = 0
                for ck in range(NCHUNK):
                    ct = open_chunk(ck, tile_base)
                    tile_base += ct["ntiles"]
                    tmap = {}
                    if ck < last:
                        chunk_tiles(conv, ck, 0, NBLK, tables, dep, ct, tmap, Fa)
                    else:
                        ctl = ct
                tmap = {}
                prods = [] if conv < 3 else None
                sdmas = []
                for q in range(NQ):
                    b0, b1 = QB0[q], QB0[q] + QBLK[q]
                    chunk_tiles(conv, last, b0, b1, tables, dep, ctl, tmap, Fa)
                    if conv < 3:
                        transform_q(conv, q)
                        sdmas.append(emit_q(hsrc_for_emit, ccp_in, q))
                if conv < 3:
                    with tc.high_priority():
                        ags = [launch_ag(ccp_in, agos, p_out_q, q)
                               for q in range(NQ)]
                    if KFP8:
                        for q in range(NQ):
                            prods.append(launch_expand(
                                agos, p_out_q, q, ags[q],
                                sdmas[min(q + 1, NQ - 1)]))
                    else:
                        prods.extend(ags)
                return prods

            STOP = int(os.environ.get("KSTOP", "9"))
            KREP = KREP0

            def bail():
                nc.sync.dma_start(out_d[0:32, :], aggT[0:32, 0:OUT_D])

            xs_tables = [xs_d[CHOFF[q]: CHOFF[q] + CHUNKROWS[q], :]
                         for q in range(NQ)]
            for _rep in range(KREP):
                # ---- conv1: tables = xs (constant)
                ag1 = conv_pass(1, xs_tables, None, h1T, ccp_in1,
                                ago1[_rep] if KFP8 else None, p1_fulls[_rep])
                if STOP <= 3:
                    bail()
                    return nc
                # ---- conv2
                ag2 = conv_pass(2, [t[:] for t in p1_fulls[_rep]], ag1, hXT,
                                ccp_in2, ago2[_rep] if KFP8 else None,
                                p2_fulls[_rep])
                if STOP <= 5:
                    bail()
                    return nc
                # ---- conv3 aggregation only
                conv_pass(3, [t[:] for t in p2_fulls[_rep]], ag2, None, None,
                          None, None)

            # ---- conv3 transform + pooling (once; outside reps)
            for b in range(NBLK):
                w = min(BLK, NLOC - b * BLK)
                bs = slice(b * BLK, b * BLK + w)
                bsl = lnpool.tile([128, 64], bf16, tag="bsl")
                nc.sync.dma_start(bsl[0:w, :], bone_d[b * BLK:b * BLK + w, :])
                dbc = auxps.tile([128, 512], f32, tag="aux")
                nc.tensor.matmul(dbc[:, 0:w], onesr[:],
                                 dinvr[0:1, bs], start=True, stop=True)
                z = lnpool.tile([128, 512], bf16, tag="z")
                nc.vector.tensor_tensor(z[:, 0:w], aggT[:, bs], dbc[:, 0:w],
                                        op.mult)
                ynm = lnpool.tile([128, 256], bf16, tag="ynm")
                for half, (wcl, bc) in enumerate(((W3ac, B3A), (W3bc, B3B))):
                    ps = mainps.tile([128, 512], f32, tag="main")
                    nc.tensor.matmul(ps[:, 0:w], wts[:, wcl], z[:, 0:w],
                                     start=True, stop=True)
                    ya = lnpool.tile([128, 512], bf16, tag="y")
                    nc.scalar.activation(ya[:, 0:w], ps[:, 0:w], AF.Identity,
                                         bias=wc[:, bc:bc + 1])
                    tp = auxps.tile([128, 128], bf16, tag="aux")
                    nc.tensor.transpose(tp[0:w, :], ya[:, 0:w], ident)
                    nc.vector.tensor_copy(ynm[0:w, half * 128:(half + 1) * 128],
                                          tp[0:w, :])
                nc.tensor.matmul(pool_ps[:], bsl[0:w, :], ynm[0:w, :],
                                 start=(b == 0), stop=(b == NBLK - 1))
            pool_sb = cpool.tile([64, OUT_D], f32)
            nc.vector.tensor_copy(pool_sb[:], pool_ps[:])
            nc.sync.dma_start(ccq_in[:], pool_sb[:])
            nc.gpsimd.collective_compute(
                "AllReduce", op.add, replica_groups=rg,
                ins=[ccq_in.opt()], outs=[ccq_out.opt()])
            par = cpool.tile([64, OUT_D], f32)
            nc.sync.dma_start(par[:], ccq_out[:])
            osb = cpool.tile([64, OUT_D], f32)
            nc.vector.tensor_scalar(osb[:], par[:], recip[:], None, op.mult)
            nc.sync.dma_start(out_d[:], osb[:])
    return nc


# ---------------- host wrapper -------------------------------------------

_CACHE = {}
_last_in_maps = None


def kernel(x, edge_index, batch, W1, b1, W2, b2, W3, b3, res_W, res_b,
           ln1_g, ln1_b, ln2_g, ln2_b):
    from concourse.bass_utils import run_bass_kernel_spmd

    x = np.asarray(x, F32)
    edge_index = np.asarray(edge_index).astype(np.int64)
    batch = np.asarray(batch).astype(np.int64)

    deg = np.bincount(
        np.concatenate([edge_index[1], np.arange(N, dtype=np.int64)]),
        minlength=N).astype(F32)
    dinv = (1.0 / np.sqrt(deg)).astype(F32)

    plans, ntiles, EP = build_plan(edge_index)

    TDT = F8 if KFP8 else BF
    TW = 256 if KFP8 else 128
    # conv1 table: x*dinv padded into [NP, TW]
    xs = np.zeros((NP, TW), F32)
    rows, _, _ = _table_row(np.arange(N, dtype=np.int64))
    xs[rows, :IN_D] = x * dinv[:, None]
    xs = xs.astype(TDT)

    # weights
    wts = np.zeros((128, 640), F32)
    wts[:, 0:128] = W2
    wts[:, 128:256] = W3[:, 0:128]
    wts[:, 256:384] = W3[:, 128:256]
    wts[:IN_D, 384:512] = W1
    wts[:IN_D, 512:640] = res_W
    wts = wts.astype(BF)

    wc = np.zeros((128, 10), F32)
    wc[:, 0] = b1 + res_b
    wc[:, 1], wc[:, 2] = ln1_g, ln1_b
    wc[:, 3], wc[:, 4], wc[:, 5] = b2, ln2_g, ln2_b
    wc[:, 6], wc[:, 7] = b3[0:128], b3[128:256]
    wc[:, 8] = 1.0
    wc[:, 9] = EPS

    misc = np.zeros((128, 257), F32)
    misc[:, 0:128] = np.arange(128, dtype=F32)[None, :]
    misc[:, 128:256] = np.eye(128, dtype=F32)
    misc[:, 256] = 1.0
    misc = misc.astype(BF)

    cnt = np.bincount(batch, minlength=B).astype(F32)
    recip = (1.0 / np.maximum(cnt, 1.0)).reshape(64, 1)

    in_maps = []
    for c in range(NCORE):
        nsl = slice(c * NLOC, (c + 1) * NLOC)
        xloc = np.zeros((32, NLOC), F32)
        xloc[:IN_D] = x[nsl].T
        dtmp = np.zeros(NLOCP, F32)
        dtmp[:NLOC] = dinv[nsl]
        dinvc = np.ascontiguousarray(dtmp.reshape(NBLK, BLK).T)
        bone = np.zeros((NLOCP, 64), F32)
        bone[np.arange(NLOC), batch[nsl]] = 1.0
        in_maps.append({
            "xs": xs, "idx": plans[c]["idx"], "dl": plans[c]["dl"],
            "wts": wts, "wconst": wc, "misc": misc,
            "xloc": xloc.astype(BF),
            "dinvf": np.ascontiguousarray(dinv[nsl]).reshape(1, NLOC).astype(BF),
            "dinvc": dinvc,
            "bone": bone.astype(BF), "recip": recip,
            "onesrow": np.ones((1, 128), BF),
        })

    global _last_in_maps
    _last_in_maps = in_maps
    key = (os.environ.get("KSTOP", "9"), KFP8, ntiles.tobytes())
    if key not in _CACHE:
        t0 = time.time()
        nc = build_nc(ntiles)
        print(f"[kernel] traced in {time.time()-t0:.1f}s", file=sys.stderr)
        t0 = time.time()
        nc.compile()
        print(f"[kernel] bacc-compiled in {time.time()-t0:.1f}s", file=sys.stderr)
        _CACHE[key] = nc
    nc = _CACHE[key]

    t0 = time.time()
    trace = bool(int(os.environ.get("KTRACE", "0")))
    res = run_bass_kernel_spmd(nc, in_maps, core_ids=list(range(NCORE)),
                               trace=trace)
    print(f"[kernel] ran in {time.time()-t0:.1f}s", file=sys.stderr)
    kernel.last_results = res
    return np.asarray(res.results[0]["out"], F32)
